# revision 1
# baseline (speedup 1.0000x reference)
"""Trainium2 Bass kernel for HHNodeMP message passing.

Reference computation (per row n of N=100000, d=256):
    node_fea = cur @ Wn
    spa_fea  = spa @ Ws
    tmp_fea  = tmp @ Wt
    s[n] = (spa_fea[n] . node_fea[n]) / 16
    t[n] = (tmp_fea[n] . node_fea[n]) / 16
    out  = relu((s*spa_fea + t*tmp_fea) @ theta_w.T + theta_b)

Algebraic restructuring (weight products precomputed on host):
    s[n] = rowsum((spa @ (Ws Wn^T)) * cur) / 16
    t[n] = rowsum((tmp @ (Wt Wn^T)) * cur) / 16
    out  = relu(s * (spa @ Ws theta_w^T) + t * (tmp @ Wt theta_w^T) + b)
so on-device only TWO matmuls per 128-row tile are needed, each with a
concatenated [256, 512] weight:
    spa @ [Ms | Wsp]   -> [q_s | g1]
    tmp @ [Mt | Wtp]   -> [q_t | g2]
`cur` is only used element-wise (fused multiply-reduce on the vector engine).

Layout strategy (all arranged on host, where it is free):
  * spa/tmp are stored PRE-TRANSPOSED per 128x128 block so the PE's
    stationary operand streams straight from DMA - no on-device
    transposes, no PSUM->SBUF copies.
  * all activations travel as fp16 (PE accumulates in fp32 PSUM), which
    halves HBM traffic; output is stored fp16 and upcast on host.
  * per core the 98 row-tiles are grouped into 7 supertiles of 14; each
    supertile's spa^T/tmp^T/cur are packed into ONE contiguous DRAM
    region so every input DMA moves 2.75 MB with 21.5 KB contiguous per
    partition (DMA efficiency needs >=1 MB transfers).

Sharding: row-parallel across 8 NeuronCores, 12544 rows/core (zero-padded
from 12500), weights replicated. No communication.
"""

import sys

import numpy as np

sys.path.insert(0, "/opt/trn_rl_repo")

import concourse.bass as bass  # noqa: E402
import concourse.mybir as mybir  # noqa: E402
import concourse.tile as tile  # noqa: E402
from concourse import bacc  # noqa: E402
from concourse.bass_utils import run_bass_kernel_spmd  # noqa: E402

N = 100000
D = 256
N_CORES = 8
ROWS_RAW = N // N_CORES            # 12500
TILES = (ROWS_RAW + 127) // 128    # 98
ROWS = TILES * 128                 # 12544
G = 14                             # tiles per supertile
S = TILES // G                     # 7 supertiles
B = 7                              # tiles per batched post-processing group
SEG = G * 2 * 128                  # 3584 fp16 elems per segment per partition
PK_W = 3 * SEG                     # packed width: spaT | tmpT | cur
OUT_W = G * D                      # 3584
F32 = mybir.dt.float32
F16 = mybir.dt.float16
BF16 = mybir.dt.bfloat16
INV_SQRT_D = 1.0 / 16.0

_CACHED_NC = None


def _build_nc(repeat: int = 1, parts: str = "full") -> bass.Bass:
    from contextlib import nullcontext

    nc = bacc.Bacc("TRN2", target_bir_lowering=False, debug=False)

    pk_d = nc.declare_dram_parameter("packed", [128, S * PK_W], F16, isOutput=False)
    ws_d = nc.declare_dram_parameter("w_scat", [128, 2, 2 * D], F16, isOutput=False)
    wt_d = nc.declare_dram_parameter("w_tcat", [128, 2, 2 * D], F16, isOutput=False)
    bf_d = nc.declare_dram_parameter("b_full", [128, B * D], BF16, isOutput=False)
    out_d = nc.declare_dram_parameter("out", [128, S * OUT_W], BF16, isOutput=True)

    with tile.TileContext(nc) as tc:
        with (
            tc.tile_pool(name="const", bufs=1) as cpool,
            tc.tile_pool(name="io", bufs=2) as iopool,
            tc.tile_pool(name="ot", bufs=2) as opool,
            tc.tile_pool(name="scr", bufs=4) as spool,
            tc.tile_pool(name="attn", bufs=8) as apool,
            tc.tile_pool(name="psum", bufs=4, space="PSUM") as ppool,
        ):
            # Constants, loaded once. Weight tiles laid out [128, kchunk, 512]
            # so chunk c / partition p holds weight row c*128+p.
            w_s = cpool.tile([128, 2, 2 * D], F16)
            nc.sync.dma_start(w_s[:], ws_d[:])
            w_t = cpool.tile([128, 2, 2 * D], F16)
            nc.sync.dma_start(w_t[:], wt_d[:])
            b_full = cpool.tile([128, B * D], BF16)
            nc.sync.dma_start(b_full[:], bf_d[:])

            # repeat>1 is a benchmarking aid (timing by differencing against
            # repeat=1): the body is idempotent, so looping it R times just
            # rewrites the same outputs.
            loop = tc.For_i(0, repeat) if repeat > 1 else nullcontext()
            with loop:
                _emit_body(nc, tc, cpool, iopool, opool, spool, apool, ppool,
                           pk_d, out_d, w_s, w_t, b_full, parts)

    nc.compile()
    return nc


def _emit_body(nc, tc, cpool, iopool, opool, spool, apool, ppool,
               pk_d, out_d, w_s, w_t, b_full, parts="full"):
    # parts: benchmarking aid to localize the bottleneck by differencing.
    #   "full"  - the real kernel
    #   "empty" - one tiny op (measures For_i loop overhead)
    #   "dma"   - input loads + output stores only
    #   "pe"    - loads + matmuls + stores (no vector/scalar)
    if parts == "empty":
        z = spool.tile([128, 1], F32)
        nc.scalar.copy(z[:], b_full[:, 0:1])
        return
    if True:
        if True:
            for s in range(S):
                pk = iopool.tile([128, PK_W], F16)
                nc.sync.dma_start(pk[:], pk_d[:, s * PK_W : (s + 1) * PK_W])
                ot = opool.tile([128, OUT_W], BF16)
                if parts in ("dma", "pe", "rs", "rsna", "rsc"):
                    # bench-only: ot must be written before the store DMA
                    nc.gpsimd.memset(ot[:], 0.0)

                for bb in range(G // B):
                    # pre_s/pre_t collect the scaled projections for B tiles
                    # so the adds + relu run batched (one big instruction
                    # instead of B small ones - per-instruction overhead on
                    # DVE/ACT dominates at [128,256] granularity).
                    pre_s = spool.tile([128, B, D], BF16)
                    pre_t = spool.tile([128, B, D], BF16)

                    for i in range(B):
                        t = bb * B + i
                        if parts == "dma":
                            continue
                        # Concatenated projections: [q_s | g1] and [q_t | g2].
                        # Stationary operand = pre-transposed 128x128 block
                        # of spa/tmp straight out of the packed DMA.
                        ps_s = ppool.tile([128, 2 * D], F32)
                        ps_t = ppool.tile([128, 2 * D], F32)
                        for c in range(2):
                            a_off = c * (G * 128) + t * 128
                            nc.tensor.matmul(
                                ps_s[:],
                                pk[:, a_off : a_off + 128],
                                w_s[:, c],
                                start=(c == 0),
                                stop=(c == 1),
                            )
                            nc.tensor.matmul(
                                ps_t[:],
                                pk[:, SEG + a_off : SEG + a_off + 128],
                                w_t[:, c],
                                start=(c == 0),
                                stop=(c == 1),
                            )

                        if parts == "pe":
                            continue

                        cur_ap = pk[:, 2 * SEG + t * D : 2 * SEG + (t + 1) * D]

                        # Fused row-dot on DVE: attn = rowsum((q/sqrt(d))*cur)
                        s_attn = apool.tile([128, 1], F32)
                        t_attn = apool.tile([128, 1], F32)
                        scr_s = spool.tile([128, D], F16)
                        scr_t = spool.tile([128, D], F16)
                        nc.vector.scalar_tensor_tensor(
                            out=scr_s[:],
                            in0=ps_s[:, 0:D],
                            scalar=INV_SQRT_D,
                            in1=cur_ap,
                            op0=mybir.AluOpType.mult,
                            op1=mybir.AluOpType.mult,
                            accum_out=s_attn[:],
                        )
                        nc.vector.scalar_tensor_tensor(
                            out=scr_t[:],
                            in0=ps_t[:, 0:D],
                            scalar=INV_SQRT_D,
                            in1=cur_ap,
                            op0=mybir.AluOpType.mult,
                            op1=mybir.AluOpType.mult,
                            accum_out=t_attn[:],
                        )
                        if parts in ("rs", "rsna"):
                            continue

                        # Per-partition scales on the scalar engine (reads
                        # PSUM, applies [128,1] scale, frees the psum bank).
                        # parts=nodep (bench-only): constant scale to break
                        # the DVE->ACT dependency and measure its latency.
                        sc_s = 0.5 if parts == "nodep" else s_attn[:]
                        sc_t = 0.5 if parts == "nodep" else t_attn[:]
                        nc.scalar.activation(
                            pre_s[:, i],
                            ps_s[:, D : 2 * D],
                            mybir.ActivationFunctionType.Copy,
                            scale=sc_s,
                        )
                        nc.scalar.activation(
                            pre_t[:, i],
                            ps_t[:, D : 2 * D],
                            mybir.ActivationFunctionType.Copy,
                            scale=sc_t,
                        )

                    if parts in ("dma", "pe", "rs", "rsna", "rsc"):
                        continue

                    # Batched: out = relu(pre_s + pre_t + b) over B tiles.
                    # add1 + relu run on the otherwise-idle GPSIMD engine;
                    # add2 on DVE in bf16 (2x mode needs bf16 specifically).
                    sum2 = spool.tile([128, B, D], BF16)
                    nc.vector.tensor_tensor(
                        out=sum2[:],
                        in0=pre_s[:],
                        in1=pre_t[:],
                        op=mybir.AluOpType.add,
                    )
                    sum3 = spool.tile([128, B, D], BF16)
                    nc.vector.tensor_tensor(
                        out=sum3[:],
                        in0=sum2[:],
                        in1=b_full[:].rearrange("p (b d) -> p b d", b=B),
                        op=mybir.AluOpType.add,
                    )
                    nc.vector.tensor_scalar_max(
                        ot[:, bb * B * D : (bb + 1) * B * D],
                        sum3[:].rearrange("p b d -> p (b d)"),
                        0.0,
                    )

                nc.sync.dma_start(out_d[:, s * OUT_W : (s + 1) * OUT_W], ot[:])


def _get_nc() -> bass.Bass:
    global _CACHED_NC
    if _CACHED_NC is None:
        _CACHED_NC = _build_nc()
    return _CACHED_NC


def _prep_inputs(
    cur, spatial_hyperedge_emb, temporal_hyperedge_emb,
    node_proj, spatial_edge_proj, temporal_edge_proj, theta_w, theta_b,
):
    cur = np.asarray(cur, np.float32)
    spa = np.asarray(spatial_hyperedge_emb, np.float32)
    tmp = np.asarray(temporal_hyperedge_emb, np.float32)
    wn = np.asarray(node_proj, np.float64)
    ws = np.asarray(spatial_edge_proj, np.float64)
    wt = np.asarray(temporal_edge_proj, np.float64)
    th = np.asarray(theta_w, np.float64)
    b = np.asarray(theta_b, np.float32)

    def wpack(m):
        # [256, 512] -> [128, 2, 512]; chunk c / partition p = weight row c*128+p
        return np.ascontiguousarray(
            m.astype(np.float32).astype(np.float16).reshape(2, 128, 2 * D).transpose(1, 0, 2)
        )

    w_scat = wpack(np.concatenate([ws @ wn.T, ws @ th.T], axis=1))
    w_tcat = wpack(np.concatenate([wt @ wn.T, wt @ th.T], axis=1))
    import ml_dtypes
    b_full = np.ascontiguousarray(np.tile(b.astype(ml_dtypes.bfloat16)[None, :], (128, B)))

    pad = N_CORES * ROWS - N
    def shard(x):
        x = np.concatenate([x, np.zeros((pad, D), np.float32)], axis=0)
        return x.reshape(N_CORES, ROWS, D).astype(np.float16)

    cur_s, spa_s, tmp_s = shard(cur), shard(spa), shard(tmp)

    def pack_T(x):
        # [ROWS, 256] -> [128p, S, 2c, G t, 128 r] -> [128, S*SEG]
        # element (p, s, c, t, r) = x[(s*G+t)*128 + r, c*128 + p]
        return x.reshape(S, G, 128, 2, 128).transpose(4, 0, 3, 1, 2).reshape(128, S * SEG)

    def pack_R(x):
        # [ROWS, 256] -> [128 r, S, G t, 256 f] -> [128, S*G*256]
        return x.reshape(S, G, 128, D).transpose(2, 0, 1, 3).reshape(128, S * G * D)

    in_maps = []
    for c in range(N_CORES):
        segs = (
            pack_T(spa_s[c]).reshape(128, S, SEG),
            pack_T(tmp_s[c]).reshape(128, S, SEG),
            pack_R(cur_s[c]).reshape(128, S, SEG),
        )
        packed = np.concatenate(segs, axis=2).reshape(128, S * PK_W)
        in_maps.append(
            {
                "packed": np.ascontiguousarray(packed),
                "w_scat": w_scat,
                "w_tcat": w_tcat,
                "b_full": b_full,
            }
        )
    return in_maps


def _unpack_out(res) -> np.ndarray:
    outs = []
    for c in range(N_CORES):
        o = res.results[c]["out"]  # [128, S*OUT_W] bf16
        o = o.reshape(128, S, G, D).transpose(1, 2, 0, 3).reshape(ROWS, D)
        outs.append(o)
    full = np.concatenate(outs, axis=0)[:N]
    return np.ascontiguousarray(full.astype(np.float32))


def kernel(**inputs) -> np.ndarray:
    in_maps = _prep_inputs(**inputs)
    nc = _get_nc()
    res = run_bass_kernel_spmd(nc, in_maps, list(range(N_CORES)))
    return _unpack_out(res)



# revision 3
# speedup vs baseline: 1.0196x; 1.0196x over previous
"""Trainium2 Bass kernel for HHNodeMP message passing.

Reference computation (per row n of N=100000, d=256):
    node_fea = cur @ Wn
    spa_fea  = spa @ Ws
    tmp_fea  = tmp @ Wt
    s[n] = (spa_fea[n] . node_fea[n]) / 16
    t[n] = (tmp_fea[n] . node_fea[n]) / 16
    out  = relu((s*spa_fea + t*tmp_fea) @ theta_w.T + theta_b)

Algebraic restructuring (weight products precomputed on host):
    s[n] = rowsum((spa @ (Ws Wn^T)) * cur) / 16
    t[n] = rowsum((tmp @ (Wt Wn^T)) * cur) / 16
    out  = relu(s * (spa @ Ws theta_w^T) + t * (tmp @ Wt theta_w^T) + b)
so on-device only TWO matmuls per 128-row tile are needed, each with a
concatenated [256, 512] weight:
    spa @ [Ms | Wsp]   -> [q_s | g1]
    tmp @ [Mt | Wtp]   -> [q_t | g2]
`cur` is only used element-wise (fused multiply-reduce on the vector engine).

Layout strategy (all arranged on host, where it is free):
  * spa/tmp are stored PRE-TRANSPOSED per 128x128 block so the PE's
    stationary operand streams straight from DMA - no on-device
    transposes, no PSUM->SBUF copies.
  * all activations travel as fp16 (PE accumulates in fp32 PSUM), which
    halves HBM traffic; output is stored fp16 and upcast on host.
  * per core the 98 row-tiles are grouped into 7 supertiles of 14; each
    supertile's spa^T/tmp^T/cur are packed into ONE contiguous DRAM
    region so every input DMA moves 2.75 MB with 21.5 KB contiguous per
    partition (DMA efficiency needs >=1 MB transfers).

Sharding: row-parallel across 8 NeuronCores, 12544 rows/core (zero-padded
from 12500), weights replicated. No communication.
"""

import sys

import numpy as np

sys.path.insert(0, "/opt/trn_rl_repo")

import concourse.bass as bass  # noqa: E402
import concourse.mybir as mybir  # noqa: E402
import concourse.tile as tile  # noqa: E402
from concourse import bacc  # noqa: E402
from concourse.bass_utils import run_bass_kernel_spmd  # noqa: E402

N = 100000
D = 256
N_CORES = 8
ROWS_RAW = N // N_CORES            # 12500
TILES = (ROWS_RAW + 127) // 128    # 98
ROWS = TILES * 128                 # 12544
G = 14                             # tiles per supertile
S = TILES // G                     # 7 supertiles
B = 7                              # tiles per batched post-processing group
SEG = G * 2 * 128                  # 3584 fp16 elems per segment per partition
PK_W = 3 * SEG                     # packed width: spaT | tmpT | cur
OUT_W = G * D                      # 3584
HS = B * 2 * 128                   # 1792: half-supertile segment (s=0 packing)
F32 = mybir.dt.float32
F16 = mybir.dt.float16
BF16 = mybir.dt.bfloat16
INV_SQRT_D = 1.0 / 16.0

_CACHED_NC = None


def _build_nc(repeat: int = 1, parts: str = "full") -> bass.Bass:
    from contextlib import nullcontext

    nc = bacc.Bacc("TRN2", target_bir_lowering=False, debug=False)

    pk_d = nc.declare_dram_parameter("packed", [128, S * PK_W], F16, isOutput=False)
    ws_d = nc.declare_dram_parameter("w_scat", [128, 2, 2 * D], F16, isOutput=False)
    wt_d = nc.declare_dram_parameter("w_tcat", [128, 2, 2 * D], F16, isOutput=False)
    bf_d = nc.declare_dram_parameter("b_full", [128, B * D], BF16, isOutput=False)
    out_d = nc.declare_dram_parameter("out", [128, S * OUT_W], BF16, isOutput=True)

    with tile.TileContext(nc) as tc:
        with (
            tc.tile_pool(name="const", bufs=1) as cpool,
            tc.tile_pool(name="io", bufs=2) as iopool,
            tc.tile_pool(name="ot", bufs=2) as opool,
            tc.tile_pool(name="scr", bufs=4) as spool,
            tc.tile_pool(name="attn", bufs=8) as apool,
            tc.tile_pool(name="psum", bufs=4, space="PSUM") as ppool,
        ):
            # Constants, loaded once. Weight tiles laid out [128, kchunk, 512]
            # so chunk c / partition p holds weight row c*128+p.
            # Const loads ride the GPSIMD SWDGE queue so they don't
            # head-of-line block the first packed-data DMA on SP.
            w_s = cpool.tile([128, 2, 2 * D], F16)
            nc.gpsimd.dma_start(w_s[:], ws_d[:])
            w_t = cpool.tile([128, 2, 2 * D], F16)
            nc.gpsimd.dma_start(w_t[:], wt_d[:])
            b_full = cpool.tile([128, B * D], BF16)
            nc.gpsimd.dma_start(b_full[:], bf_d[:])

            # repeat>1 is a benchmarking aid (timing by differencing against
            # repeat=1): the body is idempotent, so looping it R times just
            # rewrites the same outputs.
            loop = tc.For_i(0, repeat) if repeat > 1 else nullcontext()
            with loop:
                _emit_body(nc, tc, cpool, iopool, opool, spool, apool, ppool,
                           pk_d, out_d, w_s, w_t, b_full, parts)

    nc.compile()
    return nc


def _emit_body(nc, tc, cpool, iopool, opool, spool, apool, ppool,
               pk_d, out_d, w_s, w_t, b_full, parts="full"):
    # parts: benchmarking aid to localize the bottleneck by differencing.
    #   "full"  - the real kernel
    #   "empty" - one tiny op (measures For_i loop overhead)
    #   "dma"   - input loads + output stores only
    #   "pe"    - loads + matmuls + stores (no vector/scalar)
    if parts == "empty":
        z = spool.tile([128, 1], F32)
        nc.scalar.copy(z[:], b_full[:, 0:1])
        return
    if True:
        if True:
            for s in range(S):
                pk = iopool.tile([128, PK_W], F16)
                if s == 0:
                    # Supertile 0 is packed group-interleaved on the host:
                    # [spaT|tmpT|cur for tiles 0-6] then [.. for tiles 7-13],
                    # loaded as six ~460KB chunks. Tile 0's full dependency
                    # set (matmul blocks AND cur for the row-dot) lands after
                    # ~1.4MB instead of the whole 2.75MB -> ~5us less fill.
                    for seg in range(6):
                        nc.sync.dma_start(
                            pk[:, seg * HS : (seg + 1) * HS],
                            pk_d[:, seg * HS : (seg + 1) * HS],
                        )
                else:
                    nc.sync.dma_start(pk[:], pk_d[:, s * PK_W : (s + 1) * PK_W])
                ot = opool.tile([128, OUT_W], BF16)
                if parts in ("dma", "pe", "rs", "rsna", "rsc"):
                    # bench-only: ot must be written before the store DMA
                    nc.gpsimd.memset(ot[:], 0.0)

                for bb in range(G // B):
                    # pre_s/pre_t collect the scaled projections for B tiles
                    # so the adds + relu run batched (one big instruction
                    # instead of B small ones - per-instruction overhead on
                    # DVE/ACT dominates at [128,256] granularity).
                    pre_s = spool.tile([128, B, D], BF16)
                    pre_t = spool.tile([128, B, D], BF16)

                    for i in range(B):
                        t = bb * B + i
                        if parts == "dma":
                            continue
                        # Concatenated projections: [q_s | g1] and [q_t | g2].
                        # Stationary operand = pre-transposed 128x128 block
                        # of spa/tmp straight out of the packed DMA.
                        ps_s = ppool.tile([128, 2 * D], F32)
                        ps_t = ppool.tile([128, 2 * D], F32)
                        for c in range(2):
                            if s == 0:
                                base = bb * 3 * HS
                                sp_off = base + c * (B * 128) + i * 128
                                tm_off = sp_off + HS
                            else:
                                a_off = c * (G * 128) + t * 128
                                sp_off = a_off
                                tm_off = SEG + a_off
                            nc.tensor.matmul(
                                ps_s[:],
                                pk[:, sp_off : sp_off + 128],
                                w_s[:, c],
                                start=(c == 0),
                                stop=(c == 1),
                            )
                            nc.tensor.matmul(
                                ps_t[:],
                                pk[:, tm_off : tm_off + 128],
                                w_t[:, c],
                                start=(c == 0),
                                stop=(c == 1),
                            )

                        if parts == "pe":
                            continue

                        if s == 0:
                            cb = bb * 3 * HS + 2 * HS + i * D
                        else:
                            cb = 2 * SEG + t * D
                        cur_ap = pk[:, cb : cb + D]

                        # Fused row-dot on DVE: attn = rowsum((q/sqrt(d))*cur)
                        s_attn = apool.tile([128, 1], F32)
                        t_attn = apool.tile([128, 1], F32)
                        scr_s = spool.tile([128, D], F16)
                        scr_t = spool.tile([128, D], F16)
                        nc.vector.scalar_tensor_tensor(
                            out=scr_s[:],
                            in0=ps_s[:, 0:D],
                            scalar=INV_SQRT_D,
                            in1=cur_ap,
                            op0=mybir.AluOpType.mult,
                            op1=mybir.AluOpType.mult,
                            accum_out=s_attn[:],
                        )
                        nc.vector.scalar_tensor_tensor(
                            out=scr_t[:],
                            in0=ps_t[:, 0:D],
                            scalar=INV_SQRT_D,
                            in1=cur_ap,
                            op0=mybir.AluOpType.mult,
                            op1=mybir.AluOpType.mult,
                            accum_out=t_attn[:],
                        )
                        if parts in ("rs", "rsna"):
                            continue

                        # Per-partition scales on the scalar engine (reads
                        # PSUM, applies [128,1] scale, frees the psum bank).
                        # parts=nodep (bench-only): constant scale to break
                        # the DVE->ACT dependency and measure its latency.
                        sc_s = 0.5 if parts == "nodep" else s_attn[:]
                        sc_t = 0.5 if parts == "nodep" else t_attn[:]
                        nc.scalar.activation(
                            pre_s[:, i],
                            ps_s[:, D : 2 * D],
                            mybir.ActivationFunctionType.Copy,
                            scale=sc_s,
                        )
                        nc.scalar.activation(
                            pre_t[:, i],
                            ps_t[:, D : 2 * D],
                            mybir.ActivationFunctionType.Copy,
                            scale=sc_t,
                        )

                    if parts in ("dma", "pe", "rs", "rsna", "rsc"):
                        continue

                    # Batched: out = relu(pre_s + pre_t + b) over B tiles.
                    # add1 + relu run on the otherwise-idle GPSIMD engine;
                    # add2 on DVE in bf16 (2x mode needs bf16 specifically).
                    sum2 = spool.tile([128, B, D], BF16)
                    nc.vector.tensor_tensor(
                        out=sum2[:],
                        in0=pre_s[:],
                        in1=pre_t[:],
                        op=mybir.AluOpType.add,
                    )
                    sum3 = spool.tile([128, B, D], BF16)
                    nc.vector.tensor_tensor(
                        out=sum3[:],
                        in0=sum2[:],
                        in1=b_full[:].rearrange("p (b d) -> p b d", b=B),
                        op=mybir.AluOpType.add,
                    )
                    nc.scalar.activation(
                        ot[:, bb * B * D : (bb + 1) * B * D],
                        sum3[:].rearrange("p b d -> p (b d)"),
                        mybir.ActivationFunctionType.Relu,
                    )

                nc.sync.dma_start(out_d[:, s * OUT_W : (s + 1) * OUT_W], ot[:])


def _get_nc() -> bass.Bass:
    global _CACHED_NC
    if _CACHED_NC is None:
        _CACHED_NC = _build_nc()
    return _CACHED_NC


def _prep_inputs(
    cur, spatial_hyperedge_emb, temporal_hyperedge_emb,
    node_proj, spatial_edge_proj, temporal_edge_proj, theta_w, theta_b,
):
    cur = np.asarray(cur, np.float32)
    spa = np.asarray(spatial_hyperedge_emb, np.float32)
    tmp = np.asarray(temporal_hyperedge_emb, np.float32)
    wn = np.asarray(node_proj, np.float64)
    ws = np.asarray(spatial_edge_proj, np.float64)
    wt = np.asarray(temporal_edge_proj, np.float64)
    th = np.asarray(theta_w, np.float64)
    b = np.asarray(theta_b, np.float32)

    def wpack(m):
        # [256, 512] -> [128, 2, 512]; chunk c / partition p = weight row c*128+p
        return np.ascontiguousarray(
            m.astype(np.float32).astype(np.float16).reshape(2, 128, 2 * D).transpose(1, 0, 2)
        )

    w_scat = wpack(np.concatenate([ws @ wn.T, ws @ th.T], axis=1))
    w_tcat = wpack(np.concatenate([wt @ wn.T, wt @ th.T], axis=1))
    import ml_dtypes
    b_full = np.ascontiguousarray(np.tile(b.astype(ml_dtypes.bfloat16)[None, :], (128, B)))

    pad = N_CORES * ROWS - N
    def shard(x):
        x = np.concatenate([x, np.zeros((pad, D), np.float32)], axis=0)
        return x.reshape(N_CORES, ROWS, D).astype(np.float16)

    cur_s, spa_s, tmp_s = shard(cur), shard(spa), shard(tmp)

    def pack_T(x):
        # [ROWS, 256] -> [128p, S, 2c, G t, 128 r] -> [128, S*SEG]
        # element (p, s, c, t, r) = x[(s*G+t)*128 + r, c*128 + p]
        return x.reshape(S, G, 128, 2, 128).transpose(4, 0, 3, 1, 2).reshape(128, S * SEG)

    def pack_R(x):
        # [ROWS, 256] -> [128 r, S, G t, 256 f] -> [128, S*G*256]
        return x.reshape(S, G, 128, D).transpose(2, 0, 1, 3).reshape(128, S * G * D)

    in_maps = []
    for c in range(N_CORES):
        segs = (
            pack_T(spa_s[c]).reshape(128, S, SEG),
            pack_T(tmp_s[c]).reshape(128, S, SEG),
            pack_R(cur_s[c]).reshape(128, S, SEG),
        )
        packed = np.concatenate(segs, axis=2).reshape(128, S * PK_W)
        # supertile 0 -> group-interleaved halves (matches the s==0 APs)
        p0 = packed[:, 0:PK_W]
        spa0 = p0[:, 0:SEG].reshape(128, 2, G, 128)
        tmp0 = p0[:, SEG : 2 * SEG].reshape(128, 2, G, 128)
        cur0 = p0[:, 2 * SEG : 3 * SEG].reshape(128, G, D)
        halves = []
        for h in range(2):
            halves.append(spa0[:, :, h * B : (h + 1) * B].reshape(128, HS))
            halves.append(tmp0[:, :, h * B : (h + 1) * B].reshape(128, HS))
            halves.append(cur0[:, h * B : (h + 1) * B].reshape(128, HS))
        packed = np.concatenate(
            [np.concatenate(halves, axis=1)] + [packed[:, PK_W:]], axis=1
        )
        in_maps.append(
            {
                "packed": np.ascontiguousarray(packed),
                "w_scat": w_scat,
                "w_tcat": w_tcat,
                "b_full": b_full,
            }
        )
    return in_maps


def _unpack_out(res) -> np.ndarray:
    outs = []
    for c in range(N_CORES):
        o = res.results[c]["out"]  # [128, S*OUT_W] bf16
        o = o.reshape(128, S, G, D).transpose(1, 2, 0, 3).reshape(ROWS, D)
        outs.append(o)
    full = np.concatenate(outs, axis=0)[:N]
    return np.ascontiguousarray(full.astype(np.float32))


def kernel(**inputs) -> np.ndarray:
    in_maps = _prep_inputs(**inputs)
    nc = _get_nc()
    res = run_bass_kernel_spmd(nc, in_maps, list(range(N_CORES)))
    return _unpack_out(res)

